# revision 1
# baseline (speedup 1.0000x reference)
"""Trainium2 Bass kernel for nn_LowRankDynamicConv.

Math (per sample b):
  combined = [phrase_slot[b] | eos]                       [N, 2C]
  h        = relu(combined @ W1 + b1)                     [N, 4C]
  proj     = (h @ W2 + b2) viewed as [N*C, R]             [4096, 32]
  y        = x[b] @ proj   with x[b] = context_emb[b] as  [T, N*C]
  out_k[t] = sum_j y[t + j - pad_k] @ kparam_k[:, :, j]   [T, C] for k in (1,3,5)
  out      = relu(LN(concat(out_k) @ Wo + bo))            [T, C]

This is the low-rank refactor of the reference's dense dynamic conv:
  out_k = sum_j shift_j(x) @ (proj @ kparam_k[:,:,j]) == sum_j shift_j(x @ proj) @ kparam_k
Sharding: data-parallel over batch, 2 samples per core, weights replicated.

Matmuls run in float32r (TF32-like, 1 cyc/row at free>=256; ~1.5e-4 rel err).
Contraction dims must live on SBUF partitions, so x is transposed on-chip with
PE-transposes and proj is resharded [bn,(c,r)] -> [nc, r] via a DRAM bounce.
"""
import sys

sys.path.insert(0, "/opt/trn_rl_repo")

import numpy as np

import concourse.bass as bass  # noqa: F401  (bass types used via bacc)
import concourse.mybir as mybir
import concourse.tile as tile
from concourse import bacc
from concourse.bass_utils import run_bass_kernel_spmd
from concourse.masks import make_identity

F32 = mybir.dt.float32
F32R = mybir.dt.float32r
RELU = mybir.ActivationFunctionType.Relu
SQRT = mybir.ActivationFunctionType.Sqrt

NCORES = 8
BPC = 2                    # samples per core
T, N, C, R = 1024, 16, 256, 32
NCF = N * C                # 4096 flattened (n, c) contraction dim
CH = NCF // 128            # 32 nc-chunks of 128
TQ = 4                     # t processed in 4 chunks of 256
TCHUNK = T // TQ           # 256
PAD = 2                    # max conv pad (k=5)
YW = T + 2 * PAD           # padded y width, 1028
# (kernel_size, j) pairs in feat-concat order: k1 | k3 | k5
KJ = [(1, [0]), (3, [0, 1, 2]), (5, [0, 1, 2, 3, 4])]
NJ = 9                     # total j count


def _broadcast_ap(ap, parts):
    """DMA access pattern replicating a 1D/2D DRAM tensor across `parts` partitions."""
    a = ap
    return bass.AP(tensor=a.tensor, offset=a.offset, ap=[[0, parts]] + list(a.ap))


def _build():
    nc = bacc.Bacc("TRN2", num_devices=NCORES)

    xb = nc.dram_tensor("xb", [BPC, T, NCF], F32, kind="ExternalInput")
    phrase = nc.dram_tensor("phrase", [BPC * N, C], F32, kind="ExternalInput")
    eos = nc.dram_tensor("eos", [C], F32, kind="ExternalInput")
    w1 = nc.dram_tensor("w1", [2 * C, 4 * C], F32, kind="ExternalInput")
    b1 = nc.dram_tensor("b1", [4 * C], F32, kind="ExternalInput")
    w2 = nc.dram_tensor("w2", [4 * C, C * R], F32, kind="ExternalInput")
    b2 = nc.dram_tensor("b2", [C * R], F32, kind="ExternalInput")
    kjoin = nc.dram_tensor("kjoin", [NJ, R, C], F32, kind="ExternalInput")
    wo = nc.dram_tensor("wo", [3 * C, C], F32, kind="ExternalInput")
    bo = nc.dram_tensor("bo", [C], F32, kind="ExternalInput")
    gamma = nc.dram_tensor("gamma", [C], F32, kind="ExternalInput")
    beta = nc.dram_tensor("beta", [C], F32, kind="ExternalInput")
    out = nc.dram_tensor("out", [BPC, T, C], F32, kind="ExternalOutput")

    with tile.TileContext(nc) as tc:
        with tc.tile_pool(name="keep", bufs=1) as keep, \
             tc.tile_pool(name="dram", bufs=1, space="DRAM") as dram:
            ident = keep.tile([128, 128], F32)
            make_identity(nc, ident)

            # ---- persistent small weights -------------------------------------
            # kjoin as stage-4 lhsT tiles: [r=32 part, (jj, d)] then round to f32r
            kjf = keep.tile([R, NJ, C], F32)
            nc.sync.dma_start(kjf, kjoin[:, :, :].rearrange("j r d -> r j d"))
            kjr = keep.tile([R, NJ, C], F32R)
            nc.gpsimd.tensor_copy(kjr, kjf)

            # Wo as stage-5 rhs tiles: [f%128 part, f-chunk, co]
            wof = keep.tile([128, 6, C], F32)
            nc.sync.dma_start(wof, wo[:, :].rearrange("(fc p) co -> p fc co", p=128))
            wor = keep.tile([128, 6, C], F32R)
            nc.gpsimd.tensor_copy(wor, wof)

            # LN params + output bias, broadcast across partitions
            gsb = keep.tile([128, C], F32)
            nc.sync.dma_start(gsb, _broadcast_ap(gamma[:], 128))
            bsb = keep.tile([128, C], F32)
            nc.sync.dma_start(bsb, _broadcast_ap(beta[:], 128))
            bosb = keep.tile([128, C], F32)
            nc.sync.dma_start(bosb, _broadcast_ap(bo[:], 128))

            # y^T buffers, one per sample: [r=32 part, padded t] in f32r
            zsrc = keep.tile([R, PAD], F32)
            nc.vector.memset(zsrc, 0.0)
            ysb = []
            for b in range(BPC):
                y = keep.tile([R, YW], F32R, name=f"ysb{b}")
                nc.vector.tensor_copy(y[:, 0:PAD], zsrc)
                nc.vector.tensor_copy(y[:, YW - PAD:YW], zsrc)
                ysb.append(y)

            # projL: stage-3 lhsT tiles [nc%128 part, (b, ch), r] f32r (filled below)
            projr = keep.tile([128, BPC * CH, R], F32R)

            # ---- phase A: proj = (relu([phrase|eos] @ W1 + b1)) @ W2 + b2 -----
            with tc.tile_pool(name="pA", bufs=1) as pA, \
                 tc.tile_pool(name="pAs", bufs=2) as pAs, \
                 tc.tile_pool(name="psA", bufs=2, space="PSUM") as psA:
                # combined^T [c2%128 part, ko, bn]
                phsb = pA.tile([BPC * N, C], F32)
                nc.sync.dma_start(phsb, phrase[:, :])
                eossb = pA.tile([128, 2], F32)
                nc.sync.dma_start(eossb, eos[:].rearrange("(o p) -> p o", p=128))
                combT = pA.tile([128, 4, BPC * N], F32)
                for ko in range(2):
                    pt = psA.tile([128, BPC * N], F32, tag="ph")
                    nc.tensor.transpose(pt, phsb[:, ko * 128:(ko + 1) * 128],
                                        ident[:BPC * N, :BPC * N])
                    nc.vector.tensor_copy(combT[:, ko, :], pt)
                for o in range(2):
                    nc.vector.tensor_copy(
                        combT[:, 2 + o, :],
                        eossb[:, o:o + 1].to_broadcast((128, BPC * N)))

                # W1 [c2%128, ko, m]; b1 -> [m%128, mo]
                w1sb = pA.tile([128, 4, 4 * C], F32)
                nc.sync.dma_start(w1sb, w1[:, :].rearrange("(ko p) m -> p ko m", p=128))
                b1sb = pA.tile([128, 8], F32)
                nc.sync.dma_start(b1sb, b1[:].rearrange("(mo p) -> p mo", p=128))

                # h^T [m%128 part, mo, bn] = relu(W1^T combined + b1), f32r out
                hT = pA.tile([128, 8, BPC * N], F32R)
                for mo in range(8):
                    ph = psA.tile([128, BPC * N], F32, tag="h")
                    for ko in range(4):
                        nc.tensor.matmul(ph, w1sb[:, ko, mo * 128:(mo + 1) * 128],
                                         combT[:, ko, :],
                                         start=(ko == 0), stop=(ko == 3))
                    nc.scalar.activation(out=hT[:, mo, :], in_=ph, func=RELU,
                                         bias=b1sb[:, mo:mo + 1], scale=1.0)

                # proj rows [bn, (c r)] -> DRAM scratch, 512-wide column blocks
                scratch = dram.tile([BPC * N, C * R], F32)
                for j16 in range(16):
                    w2f = pAs.tile([128, 8, 512], F32, tag="w2f")
                    nc.sync.dma_start(
                        w2f, w2[:, j16 * 512:(j16 + 1) * 512]
                        .rearrange("(ko p) q -> p ko q", p=128))
                    w2r = pAs.tile([128, 8, 512], F32R, tag="w2r")
                    nc.gpsimd.tensor_copy(w2r, w2f)
                    pp = psA.tile([BPC * N, 512], F32, tag="pj")
                    for ko in range(8):
                        nc.tensor.matmul(pp, hT[:, ko, :], w2r[:, ko, :],
                                         start=(ko == 0), stop=(ko == 7))
                    b2sb = pAs.tile([BPC * N, 512], F32, tag="b2")
                    nc.sync.dma_start(
                        b2sb, _broadcast_ap(b2[j16 * 512:(j16 + 1) * 512], BPC * N))
                    pjsb = pAs.tile([BPC * N, 512], F32, tag="pjsb")
                    nc.vector.tensor_add(pjsb, pp, b2sb)
                    nc.sync.dma_start(scratch[:, j16 * 512:(j16 + 1) * 512], pjsb)

                # reshard: [bn, (c r)] -> [c%128 part, (b, ch), r], then round
                projf = pA.tile([128, BPC * CH, R], F32, name="projf")
                for b in range(BPC):
                    for ch in range(CH):
                        row = b * N + ch // 2
                        off = (ch % 2) * 4096
                        nc.sync.dma_start(
                            projf[:, b * CH + ch, :],
                            scratch[row, off:off + 4096]
                            .rearrange("(p r) -> p r", p=128))
                nc.gpsimd.tensor_copy(projr, projf)

            # ---- phase X: per (sample, t-chunk) stream ------------------------
            with tc.tile_pool(name="pX", bufs=2) as pX, \
                 tc.tile_pool(name="pXn", bufs=3) as pXn, \
                 tc.tile_pool(name="tp", bufs=2, space="PSUM") as tp, \
                 tc.tile_pool(name="yp", bufs=2, space="PSUM") as yp, \
                 tc.tile_pool(name="fp", bufs=2, space="PSUM") as fp, \
                 tc.tile_pool(name="op", bufs=2, space="PSUM") as op:
                def emit_feat(b, tq):
                    t0 = tq * TCHUNK
                    # stage 4: feat^T[d-block, t] = sum_j kjoin_j^T @ shift(y^T)
                    featT = pX.tile([128, 6, TCHUNK], F32R, tag="featT")
                    jj = 0
                    for kb, (k, js) in enumerate(KJ):
                        pad = k // 2
                        for dc in range(2):
                            pf = fp.tile([128, TCHUNK], F32, tag="f")
                            for ji, j in enumerate(js):
                                ys = ysb[b][:, PAD + t0 + j - pad:
                                            PAD + t0 + j - pad + TCHUNK]
                                nc.tensor.matmul(
                                    pf, kjr[:, jj + ji, dc * 128:(dc + 1) * 128],
                                    ys, start=(ji == 0), stop=(ji == len(js) - 1))
                            nc.vector.tensor_copy(featT[:, kb * 2 + dc, :], pf)
                        jj += len(js)

                    # stage 5: out[t%128, co] = feat @ Wo, then LN + relu
                    for ts in range(TCHUNK // 128):
                        po = op.tile([128, C], F32, tag="o")
                        for fc in range(6):
                            nc.tensor.matmul(
                                po, featT[:, fc, ts * 128:(ts + 1) * 128],
                                wor[:, fc, :], start=(fc == 0), stop=(fc == 5))
                        osb = pX.tile([128, C], F32, tag="osb")
                        nc.vector.tensor_add(osb, po, bosb)
                        st = pX.tile([128, 6], F32, tag="st")
                        nc.vector.bn_stats(out=st, in_=osb)
                        mv = pX.tile([128, 2], F32, tag="mv")
                        nc.vector.bn_aggr(out=mv, in_=st)
                        # rstd = 1/sqrt(var + eps)
                        rs = pX.tile([128, 1], F32, tag="rs")
                        eps = pX.tile([128, 1], F32, tag="eps")
                        nc.vector.memset(eps, 1e-5)
                        nc.scalar.activation(out=rs, in_=mv[:, 1:2], func=SQRT,
                                             bias=eps, scale=1.0)
                        nc.vector.reciprocal(rs, rs)
                        nc.vector.tensor_scalar(osb, osb, mv[:, 0:1], rs,
                                                mybir.AluOpType.subtract,
                                                mybir.AluOpType.mult)
                        nc.vector.tensor_mul(osb, osb, gsb)
                        nc.vector.tensor_add(osb, osb, bsb)
                        nc.vector.tensor_scalar_max(osb, osb, 0.0)
                        nc.sync.dma_start(
                            out[b, t0 + ts * 128:t0 + (ts + 1) * 128, :], osb)

                for b in range(BPC):
                    for tq in range(TQ):
                        t0 = tq * TCHUNK
                        # x natural [t%128, nc] tiles and transposed [nc%128, ch, t]
                        xT = pX.tile([128, CH, TCHUNK], F32R, tag="xT")
                        for ts in range(TCHUNK // 128):
                            xn = pXn.tile([128, NCF], F32, tag="xn")
                            nc.sync.dma_start(
                                xn, xb[b, t0 + ts * 128:t0 + (ts + 1) * 128, :])
                            for cg in range(CH // 4):   # 4 transposes per bank
                                pt = tp.tile([128, 4, 128], F32, tag="tp")
                                for q in range(4):
                                    chx = cg * 4 + q
                                    nc.tensor.transpose(
                                        pt[:, q, :],
                                        xn[:, chx * 128:(chx + 1) * 128], ident)
                                dst = xT[:, cg * 4:(cg + 1) * 4,
                                         ts * 128:(ts + 1) * 128]
                                if cg % 2 == 0:
                                    nc.vector.tensor_copy(dst, pt)
                                else:
                                    nc.scalar.copy(dst, pt)

                        # stage 3: y^T[r, t-chunk] = sum_ch projL^T @ xT
                        py = yp.tile([R, TCHUNK], F32, tag="y")
                        for ch in range(CH):
                            nc.tensor.matmul(py, projr[:, b * CH + ch, :],
                                             xT[:, ch, :],
                                             start=(ch == 0), stop=(ch == CH - 1))
                        nc.vector.tensor_copy(ysb[b][:, PAD + t0:PAD + t0 + TCHUNK],
                                              py)
                        # stage 4/5 lag one chunk: feat(tq-1) needs y[tq]'s
                        # first PAD columns (k=5 right overhang)
                        if tq > 0:
                            emit_feat(b, tq - 1)
                    emit_feat(b, TQ - 1)

    nc.compile()
    return nc


_NC = None


def _get_nc():
    global _NC
    if _NC is None:
        _NC = _build()
    return _NC


def _shard(inputs):
    """Split full inputs into per-core input maps (pure slicing/stacking)."""
    x = np.ascontiguousarray(inputs["context_emb"], dtype=np.float32)
    B = x.shape[0]
    assert B == NCORES * BPC
    x = x.reshape(B, T, NCF)
    ph = np.ascontiguousarray(inputs["phrase_slot"], dtype=np.float32)
    kjoin = np.ascontiguousarray(np.concatenate(
        [np.moveaxis(inputs[f"k{k}"], 2, 0) for k in (1, 3, 5)], axis=0),
        dtype=np.float32)  # [9, 32, 256]
    shared = {
        "eos": np.ascontiguousarray(inputs["eos_slot"].reshape(C), dtype=np.float32),
        "w1": np.ascontiguousarray(inputs["W1"], dtype=np.float32),
        "b1": np.ascontiguousarray(inputs["b1"], dtype=np.float32),
        "w2": np.ascontiguousarray(inputs["W2"], dtype=np.float32),
        "b2": np.ascontiguousarray(inputs["b2"], dtype=np.float32),
        "kjoin": kjoin,
        "wo": np.ascontiguousarray(inputs["Wo"], dtype=np.float32),
        "bo": np.ascontiguousarray(inputs["bo"], dtype=np.float32),
        "gamma": np.ascontiguousarray(inputs["gamma"], dtype=np.float32),
        "beta": np.ascontiguousarray(inputs["beta"], dtype=np.float32),
    }
    in_maps = []
    for i in range(NCORES):
        m = dict(shared)
        m["xb"] = np.ascontiguousarray(x[i * BPC:(i + 1) * BPC])
        m["phrase"] = np.ascontiguousarray(
            ph[i * BPC:(i + 1) * BPC].reshape(BPC * N, C))
        in_maps.append(m)
    return in_maps


def _run(inputs, **kwargs):
    nc = _get_nc()
    res = run_bass_kernel_spmd(nc, _shard(inputs), core_ids=list(range(NCORES)),
                               **kwargs)
    outs = [r["out"] for r in res.results]
    full = np.concatenate(outs, axis=0).reshape(NCORES * BPC, T, C)
    return full, res


def kernel(**inputs) -> np.ndarray:
    out, _ = _run(inputs)
    return out



# revision 5
# speedup vs baseline: 1.4908x; 1.4908x over previous
"""Trainium2 Bass kernel for nn_LowRankDynamicConv.

Math (per sample b):
  combined = [phrase_slot[b] | eos]                       [N, 2C]
  h        = relu(combined @ W1 + b1)                     [N, 4C]
  proj     = (h @ W2 + b2) viewed as [N*C, R]             [4096, 32]
  y        = x[b] @ proj   with x[b] = context_emb[b] as  [T, N*C]
  out_k[t] = sum_j y[t + j - pad_k] @ kparam_k[:, :, j]   [T, C] for k in (1,3,5)
  out      = relu(LN(concat(out_k) @ Wo + bo))            [T, C]

This is the low-rank refactor of the reference's dense dynamic conv:
  out_k = sum_j shift_j(x) @ (proj @ kparam_k[:,:,j]) == sum_j shift_j(x @ proj) @ kparam_k

Sharding: data-parallel over batch, 2 samples per core. W2 (the one big
weight, 32MB) is column-sharded: every core computes the proj slice for
ALL 16 samples with its 1/8 of W2's columns, then an on-chip AllToAll
redistributes so each core holds the full proj for its own 2 samples.
This cuts per-core HBM traffic by ~28MB vs replicating W2.

Precision: phase A (tiny) runs in plain f32. x is PE-transposed in f32,
then stored bf16; stage 3-5 matmuls run in bf16 (fast weight load +
1 cyc/row), accumulating in f32 PSUM. Measured end-to-end rel err ~5e-3
against the f32 reference.
"""
import sys

sys.path.insert(0, "/opt/trn_rl_repo")

import numpy as np

import concourse.bass as bass
import concourse.mybir as mybir
import concourse.tile as tile
from concourse import bacc
from concourse.bass_utils import run_bass_kernel_spmd
from concourse.masks import make_identity

F32 = mybir.dt.float32
BF16 = mybir.dt.bfloat16
RELU = mybir.ActivationFunctionType.Relu
SQRT = mybir.ActivationFunctionType.Sqrt

NCORES = 8
BPC = 2                    # samples per core
T, N, C, R = 1024, 16, 256, 32
BN = NCORES * BPC * N      # 256 (b, n) rows across ALL samples
NCF = N * C                # 4096 flattened (n, c) contraction dim
CH = NCF // 128            # 32 nc-chunks of 128
TQ = 4                     # t processed in 4 chunks of 256
TCHUNK = T // TQ           # 256
PAD = 2                    # max conv pad (k=5)
YW = T + 2 * PAD           # padded y width, 1028
W2C = C * R // NCORES      # 1024 W2 columns per core
# (kernel_size, j) pairs in feat-concat order: k1 | k3 | k5
KJ = [(1, [0]), (3, [0, 1, 2]), (5, [0, 1, 2, 3, 4])]
NJ = 9                     # total j count


def _broadcast_ap(ap, parts):
    """DMA access pattern replicating a 1D/2D DRAM tensor across `parts` partitions."""
    a = ap
    return bass.AP(tensor=a.tensor, offset=a.offset, ap=[[0, parts]] + list(a.ap))


def _build():
    nc = bacc.Bacc("TRN2", num_devices=NCORES)

    xb = nc.dram_tensor("xb", [BPC, T, NCF], F32, kind="ExternalInput")
    phrase = nc.dram_tensor("phrase", [BN, C], F32, kind="ExternalInput")
    eos = nc.dram_tensor("eos", [C], F32, kind="ExternalInput")
    w1 = nc.dram_tensor("w1", [2 * C, 4 * C], F32, kind="ExternalInput")
    b1 = nc.dram_tensor("b1", [4 * C], F32, kind="ExternalInput")
    w2s = nc.dram_tensor("w2s", [4 * C, W2C], F32, kind="ExternalInput")
    b2s = nc.dram_tensor("b2s", [W2C], F32, kind="ExternalInput")
    kjoin = nc.dram_tensor("kjoin", [NJ, R, C], F32, kind="ExternalInput")
    wo = nc.dram_tensor("wo", [3 * C, C], F32, kind="ExternalInput")
    bo = nc.dram_tensor("bo", [C], F32, kind="ExternalInput")
    gamma = nc.dram_tensor("gamma", [C], F32, kind="ExternalInput")
    beta = nc.dram_tensor("beta", [C], F32, kind="ExternalInput")
    out = nc.dram_tensor("out", [BPC, T, C], F32, kind="ExternalOutput")

    with tile.TileContext(nc) as tc:
        with tc.tile_pool(name="keep", bufs=1) as keep, \
             tc.tile_pool(name="dram", bufs=1, space="DRAM") as dram, \
             tc.tile_pool(name="tp", bufs=2, space="PSUM") as tp, \
             tc.tile_pool(name="pXn", bufs=3) as pXn, \
             tc.tile_pool(name="pX", bufs=2) as pX:
            ident = keep.tile([128, 128], F32)
            make_identity(nc, ident)
            ones1 = keep.tile([1, 128], F32)
            nc.vector.memset(ones1, 1.0)
            eps = keep.tile([128, 1], F32)
            nc.vector.memset(eps, 1e-5)

            # LN params + output bias, broadcast across partitions
            gsb = keep.tile([128, C], F32)
            nc.sync.dma_start(gsb, _broadcast_ap(gamma[:], 128))
            bsb = keep.tile([128, C], F32)
            nc.sync.dma_start(bsb, _broadcast_ap(beta[:], 128))
            bosb = keep.tile([128, C], F32)
            nc.sync.dma_start(bosb, _broadcast_ap(bo[:], 128))

            # bf16 copies of the small stage-4/5 weights
            kjb = keep.tile([R, NJ, C], BF16)
            wob = keep.tile([128, 6, C], BF16)

            # y^T buffers, one per sample: [r=32 part, padded t] in bf16
            ysb = []
            for b in range(BPC):
                y = keep.tile([R, YW], BF16, name=f"ysb{b}")
                nc.vector.memset(y[:, 0:PAD], 0.0)
                nc.vector.memset(y[:, YW - PAD:YW], 0.0)
                ysb.append(y)

            # proj for this core's samples: [nc%128 part, (b, ch), r]
            projf = keep.tile([128, BPC * CH, R], F32)
            projb = keep.tile([128, BPC * CH, R], BF16)

            # AllToAll bounce buffers: [src/dst core, bn rows, W2-col slice]
            in_b = dram.tile([NCORES, BPC * N, W2C], F32)
            out_b = dram.tile([NCORES, BPC * N, W2C], F32)

            # ---- phase A: proj slices for all samples, AllToAll, reshard ----
            with tc.tile_pool(name="pA", bufs=1) as pA, \
                 tc.tile_pool(name="pAs", bufs=2) as pAs, \
                 tc.tile_pool(name="psA", bufs=2, space="PSUM") as psA:
                kjf = pA.tile([R, NJ, C], F32)
                nc.sync.dma_start(kjf, kjoin[:, :, :].rearrange("j r d -> r j d"))
                nc.vector.tensor_copy(kjb, kjf)
                wof = pA.tile([128, 6, C], F32)
                nc.sync.dma_start(wof, wo[:, :].rearrange("(fc p) co -> p fc co", p=128))
                nc.vector.tensor_copy(wob, wof)

                # combined^T [c2%128 part, ko, bn] for ALL bn rows
                phsb = pA.tile([128, 2, C], F32)
                nc.sync.dma_start(phsb, phrase[:, :].rearrange("(rt p) c -> p rt c", p=128))
                eossb = pA.tile([128, 2], F32)
                nc.sync.dma_start(eossb, eos[:].rearrange("(o p) -> p o", p=128))
                combT = pA.tile([128, 4, BN], F32)
                for rt in range(2):
                    for ko in range(2):
                        pht = psA.tile([128, 128], F32, tag="ph")
                        nc.tensor.transpose(pht, phsb[:, rt, ko * 128:(ko + 1) * 128],
                                            ident)
                        nc.vector.tensor_copy(combT[:, ko, rt * 128:(rt + 1) * 128], pht)
                for o in range(2):
                    nc.vector.tensor_copy(
                        combT[:, 2 + o, :],
                        eossb[:, o:o + 1].to_broadcast((128, BN)))

                # W1 [c2%128, ko, m]; b1 -> [m%128, mo]
                w1sb = pA.tile([128, 4, 4 * C], F32)
                nc.sync.dma_start(w1sb, w1[:, :].rearrange("(ko p) m -> p ko m", p=128))
                b1sb = pA.tile([128, 8], F32)
                nc.sync.dma_start(b1sb, b1[:].rearrange("(mo p) -> p mo", p=128))

                # h^T [m%128 part, mo, bn] = relu(W1^T combined + b1)
                hT = pA.tile([128, 8, BN], F32)
                for mo in range(8):
                    phm = psA.tile([128, BN], F32, tag="h")
                    for ko in range(4):
                        nc.tensor.matmul(phm, w1sb[:, ko, mo * 128:(mo + 1) * 128],
                                         combT[:, ko, :],
                                         start=(ko == 0), stop=(ko == 3))
                    nc.scalar.activation(out=hT[:, mo, :], in_=phm, func=RELU,
                                         bias=b1sb[:, mo:mo + 1], scale=1.0)

                # proj slice rows [bn, W2C] for all samples; +b2 via rank-1 matmul
                w2sb = pA.tile([128, 8, W2C], F32)
                nc.sync.dma_start(w2sb, w2s[:, :].rearrange("(ko p) q -> p ko q", p=128))
                b2sb = pA.tile([1, W2C], F32)
                nc.sync.dma_start(b2sb, b2s[:].rearrange("(p q) -> p q", p=1))
                for rt in range(2):
                    for cc in range(2):
                        ppp = psA.tile([128, 512], F32, tag="pj")
                        for ko in range(8):
                            nc.tensor.matmul(
                                ppp, hT[:, ko, rt * 128:(rt + 1) * 128],
                                w2sb[:, ko, cc * 512:(cc + 1) * 512],
                                start=(ko == 0), stop=False)
                        nc.tensor.matmul(ppp, ones1[:, 0:128],
                                         b2sb[:, cc * 512:(cc + 1) * 512],
                                         start=False, stop=True)
                        ppsb = pAs.tile([128, 512], F32, tag="ppsb")
                        nc.scalar.copy(ppsb, ppp)
                        nc.scalar.dma_start(
                            in_b[rt * 4:(rt + 1) * 4, :, cc * 512:(cc + 1) * 512], ppsb)

                nc.gpsimd.collective_compute(
                    "AllToAll",
                    mybir.AluOpType.bypass,
                    replica_groups=[list(range(NCORES))],
                    ins=[in_b[:, :, :].opt()],
                    outs=[out_b[:, :, :].opt()],
                )

                # reshard: out_b[i = half*4+q, b*16+n, cl*32+r] -> projf[q*32+cl, (b,ch), r]
                # with ch = n*2 + half, i.e. nc = ch*128 + p matching xT's chunks.
                ob = out_b[:, :, :]
                for q in range(4):
                    for half in range(2):
                        src = bass.AP(
                            tensor=ob.tensor,
                            offset=ob.offset + (half * 4 + q) * (BPC * N * W2C),
                            ap=[[32, 32],        # cl -> partition within the q-group
                                [W2C, BPC * N],  # (b, n) merged
                                [1, R]])         # r
                        dst = projf[q * 32:(q + 1) * 32, :, :].rearrange(
                            "p (bn two) r -> p two bn r", two=2)[:, half, :, :]
                        nc.scalar.dma_start(dst, src)
                nc.gpsimd.tensor_copy(projb, projf)

            # ---- phase X: per (sample, t-chunk) stream ------------------------
            with tc.tile_pool(name="yp", bufs=2, space="PSUM") as yp, \
                 tc.tile_pool(name="fp", bufs=2, space="PSUM") as fp, \
                 tc.tile_pool(name="op", bufs=2, space="PSUM") as op, \
                 tc.tile_pool(name="pXs", bufs=2) as pXs:
                def emit_feat(b, tq):
                    t0 = tq * TCHUNK
                    # stage 4: feat^T[d-block, t] = sum_j kjoin_j^T @ shift(y^T)
                    featT = pX.tile([128, 6, TCHUNK], BF16, tag="featT")
                    jj = 0
                    for kb, (k, js) in enumerate(KJ):
                        pad = k // 2
                        for dc in range(2):
                            pf = fp.tile([128, TCHUNK], F32, tag="f")
                            for ji, j in enumerate(js):
                                ys = ysb[b][:, PAD + t0 + j - pad:
                                            PAD + t0 + j - pad + TCHUNK]
                                nc.tensor.matmul(
                                    pf, kjb[:, jj + ji, dc * 128:(dc + 1) * 128],
                                    ys, start=(ji == 0), stop=(ji == len(js) - 1))
                            nc.vector.tensor_copy(featT[:, kb * 2 + dc, :], pf)
                        jj += len(js)

                    # stage 5: out[t%128, co] = feat @ Wo, then LN + relu
                    for ts in range(TCHUNK // 128):
                        po = op.tile([128, C], F32, tag="o")
                        for fc in range(6):
                            nc.tensor.matmul(
                                po, featT[:, fc, ts * 128:(ts + 1) * 128],
                                wob[:, fc, :], start=(fc == 0), stop=(fc == 5))
                        osb = pXs.tile([128, C], F32, tag="osb")
                        nc.vector.tensor_add(osb, po, bosb)
                        st = pXs.tile([128, 6], F32, tag="st")
                        nc.vector.bn_stats(out=st, in_=osb)
                        mv = pXs.tile([128, 2], F32, tag="mv")
                        nc.vector.bn_aggr(out=mv, in_=st)
                        # rstd = 1/sqrt(var + eps)
                        rs = pXs.tile([128, 1], F32, tag="rs")
                        nc.scalar.activation(out=rs, in_=mv[:, 1:2], func=SQRT,
                                             bias=eps, scale=1.0)
                        nc.vector.reciprocal(rs, rs)
                        nc.vector.tensor_scalar(osb, osb, mv[:, 0:1], rs,
                                                mybir.AluOpType.subtract,
                                                mybir.AluOpType.mult)
                        nc.vector.tensor_mul(osb, osb, gsb)
                        nc.vector.tensor_add(osb, osb, bsb)
                        nc.vector.tensor_scalar_max(osb, osb, 0.0)
                        nc.sync.dma_start(
                            out[b, t0 + ts * 128:t0 + (ts + 1) * 128, :], osb)

                for b in range(BPC):
                    for tq in range(TQ):
                        chunk = b * TQ + tq
                        t0 = tq * TCHUNK
                        # x natural [t%128, nc] tiles, PE-transposed to [nc%128, ch, t]
                        xT = pX.tile([128, CH, TCHUNK], BF16, tag="xT")
                        for ts in range(TCHUNK // 128):
                            xn = pXn.tile([128, NCF], F32, tag="xn")
                            nc.sync.dma_start(
                                xn, xb[b, t0 + ts * 128:t0 + (ts + 1) * 128, :])
                            for cg in range(CH // 4):   # 4 transposes per bank
                                pt = tp.tile([128, 4, 128], F32, tag="tp")
                                for q in range(4):
                                    chx = cg * 4 + q
                                    nc.tensor.transpose(
                                        pt[:, q, :],
                                        xn[:, chx * 128:(chx + 1) * 128], ident)
                                dst = xT[:, cg * 4:(cg + 1) * 4,
                                         ts * 128:(ts + 1) * 128]
                                # first chunks: DVE only (scalar is busy with
                                # phase A's bounce DMAs until the A2A lands)
                                if chunk < 2 or cg % 2 == 0:
                                    nc.vector.tensor_copy(dst, pt)
                                else:
                                    nc.scalar.copy(dst, pt)

                        # stage 3: y^T[r, t-chunk] = sum_ch proj^T @ xT
                        py = yp.tile([R, TCHUNK], F32, tag="y")
                        for ch in range(CH):
                            nc.tensor.matmul(py, projb[:, b * CH + ch, :],
                                             xT[:, ch, :],
                                             start=(ch == 0), stop=(ch == CH - 1))
                        nc.vector.tensor_copy(ysb[b][:, PAD + t0:PAD + t0 + TCHUNK],
                                              py)
                        # stage 4/5 lag one chunk: feat(tq-1) needs y[tq]'s
                        # first PAD columns (k=5 right overhang)
                        if tq > 0:
                            emit_feat(b, tq - 1)
                    emit_feat(b, TQ - 1)

    nc.compile()
    return nc


_NC = None


def _get_nc():
    global _NC
    if _NC is None:
        _NC = _build()
    return _NC


def _shard(inputs):
    """Split full inputs into per-core input maps (pure slicing/stacking)."""
    x = np.ascontiguousarray(inputs["context_emb"], dtype=np.float32)
    B = x.shape[0]
    assert B == NCORES * BPC
    x = x.reshape(B, T, NCF)
    ph = np.ascontiguousarray(
        inputs["phrase_slot"], dtype=np.float32).reshape(BN, C)
    w2 = np.asarray(inputs["W2"], dtype=np.float32)
    b2 = np.asarray(inputs["b2"], dtype=np.float32)
    kjoin = np.ascontiguousarray(np.concatenate(
        [np.moveaxis(inputs[f"k{k}"], 2, 0) for k in (1, 3, 5)], axis=0),
        dtype=np.float32)  # [9, 32, 256]
    shared = {
        "phrase": ph,
        "eos": np.ascontiguousarray(inputs["eos_slot"].reshape(C), dtype=np.float32),
        "w1": np.ascontiguousarray(inputs["W1"], dtype=np.float32),
        "b1": np.ascontiguousarray(inputs["b1"], dtype=np.float32),
        "kjoin": kjoin,
        "wo": np.ascontiguousarray(inputs["Wo"], dtype=np.float32),
        "bo": np.ascontiguousarray(inputs["bo"], dtype=np.float32),
        "gamma": np.ascontiguousarray(inputs["gamma"], dtype=np.float32),
        "beta": np.ascontiguousarray(inputs["beta"], dtype=np.float32),
    }
    in_maps = []
    for i in range(NCORES):
        m = dict(shared)
        m["xb"] = np.ascontiguousarray(x[i * BPC:(i + 1) * BPC])
        m["w2s"] = np.ascontiguousarray(w2[:, i * W2C:(i + 1) * W2C])
        m["b2s"] = np.ascontiguousarray(b2[i * W2C:(i + 1) * W2C])
        in_maps.append(m)
    return in_maps


def _run(inputs, **kwargs):
    nc = _get_nc()
    res = run_bass_kernel_spmd(nc, _shard(inputs), core_ids=list(range(NCORES)),
                               **kwargs)
    outs = [r["out"] for r in res.results]
    full = np.concatenate(outs, axis=0).reshape(NCORES * BPC, T, C)
    return full, res


def kernel(**inputs) -> np.ndarray:
    out, _ = _run(inputs)
    return out


# revision 6
# speedup vs baseline: 1.4954x; 1.0031x over previous
"""Trainium2 Bass kernel for nn_LowRankDynamicConv.

Math (per sample b):
  combined = [phrase_slot[b] | eos]                       [N, 2C]
  h        = relu(combined @ W1 + b1)                     [N, 4C]
  proj     = (h @ W2 + b2) viewed as [N*C, R]             [4096, 32]
  y        = x[b] @ proj   with x[b] = context_emb[b] as  [T, N*C]
  out_k[t] = sum_j y[t + j - pad_k] @ kparam_k[:, :, j]   [T, C] for k in (1,3,5)
  out      = relu(LN(concat(out_k) @ Wo + bo))            [T, C]

This is the low-rank refactor of the reference's dense dynamic conv:
  out_k = sum_j shift_j(x) @ (proj @ kparam_k[:,:,j]) == sum_j shift_j(x @ proj) @ kparam_k

Sharding: data-parallel over batch, 2 samples per core. W2 (the one big
weight, 32MB) is column-sharded: every core computes the proj slice for
ALL 16 samples with its 1/8 of W2's columns, then an on-chip AllToAll
redistributes so each core holds the full proj for its own 2 samples.
This cuts per-core HBM traffic by ~28MB vs replicating W2.

Precision: weights and activations feeding matmuls are cast to bf16
(fast weight load + 1 cyc/row); accumulation stays f32 in PSUM, and the
LN epilogue is f32. x is PE-transposed in f32 and stored bf16. Measured
end-to-end rel err vs the f32 reference is well under the 2e-2 gate.
"""
import sys

sys.path.insert(0, "/opt/trn_rl_repo")

import numpy as np

import concourse.bass as bass
import concourse.mybir as mybir
import concourse.tile as tile
from concourse import bacc
from concourse.bass_utils import run_bass_kernel_spmd
from concourse.masks import make_identity

F32 = mybir.dt.float32
BF16 = mybir.dt.bfloat16
RELU = mybir.ActivationFunctionType.Relu
SQRT = mybir.ActivationFunctionType.Sqrt

NCORES = 8
BPC = 2                    # samples per core
T, N, C, R = 1024, 16, 256, 32
BN = NCORES * BPC * N      # 256 (b, n) rows across ALL samples
NCF = N * C                # 4096 flattened (n, c) contraction dim
CH = NCF // 128            # 32 nc-chunks of 128
TQ = 2                     # t processed in 2 chunks of 512
TCHUNK = T // TQ           # 512
PAD = 2                    # max conv pad (k=5)
YW = T + 2 * PAD           # padded y width, 1028
W2C = C * R // NCORES      # 1024 W2 columns per core
# (kernel_size, j) pairs in feat-concat order: k1 | k3 | k5
KJ = [(1, [0]), (3, [0, 1, 2]), (5, [0, 1, 2, 3, 4])]
NJ = 9                     # total j count


def _broadcast_ap(ap, parts):
    """DMA access pattern replicating a 1D/2D DRAM tensor across `parts` partitions."""
    a = ap
    return bass.AP(tensor=a.tensor, offset=a.offset, ap=[[0, parts]] + list(a.ap))


def _build():
    nc = bacc.Bacc("TRN2", num_devices=NCORES)

    xb = nc.dram_tensor("xb", [BPC, T, NCF], F32, kind="ExternalInput")
    phrase = nc.dram_tensor("phrase", [BN, C], F32, kind="ExternalInput")
    eos = nc.dram_tensor("eos", [C], F32, kind="ExternalInput")
    w1 = nc.dram_tensor("w1", [2 * C, 4 * C], F32, kind="ExternalInput")
    b1 = nc.dram_tensor("b1", [4 * C], F32, kind="ExternalInput")
    w2s = nc.dram_tensor("w2s", [4 * C, W2C], F32, kind="ExternalInput")
    b2s = nc.dram_tensor("b2s", [W2C], F32, kind="ExternalInput")
    kjoin = nc.dram_tensor("kjoin", [NJ, R, C], F32, kind="ExternalInput")
    wo = nc.dram_tensor("wo", [3 * C, C], F32, kind="ExternalInput")
    bo = nc.dram_tensor("bo", [C], F32, kind="ExternalInput")
    gamma = nc.dram_tensor("gamma", [C], F32, kind="ExternalInput")
    beta = nc.dram_tensor("beta", [C], F32, kind="ExternalInput")
    out = nc.dram_tensor("out", [BPC, T, C], F32, kind="ExternalOutput")

    with tile.TileContext(nc) as tc:
        with tc.tile_pool(name="keep", bufs=1) as keep, \
             tc.tile_pool(name="dram", bufs=1, space="DRAM") as dram, \
             tc.tile_pool(name="tp", bufs=2, space="PSUM") as tp, \
             tc.tile_pool(name="pXn", bufs=2) as pXn, \
             tc.tile_pool(name="pX", bufs=2) as pX:
            ident = keep.tile([128, 128], F32)
            make_identity(nc, ident)
            ones1 = keep.tile([1, 128], BF16)
            nc.vector.memset(ones1, 1.0)
            eps = keep.tile([128, 1], F32)
            nc.vector.memset(eps, 1e-5)

            # LN params + output bias, broadcast across partitions
            gsb = keep.tile([128, C], F32)
            nc.sync.dma_start(gsb, _broadcast_ap(gamma[:], 128))
            bsb = keep.tile([128, C], F32)
            nc.sync.dma_start(bsb, _broadcast_ap(beta[:], 128))
            bosb = keep.tile([128, C], F32)
            nc.sync.dma_start(bosb, _broadcast_ap(bo[:], 128))

            # bf16 stage-4/5 weights, staged through one shared f32 scratch tag
            wob = keep.tile([128, 6, C], BF16)
            wof = keep.tile([128, 6, C], F32, tag="stg", name="wof")
            nc.sync.dma_start(wof, wo[:, :].rearrange("(fc p) co -> p fc co", p=128))
            nc.vector.tensor_copy(wob, wof)
            kjb = keep.tile([R, NJ, C], BF16)
            kjf = keep.tile([R, NJ, C], F32, tag="stg", name="kjf")
            nc.sync.dma_start(kjf, kjoin[:, :, :].rearrange("j r d -> r j d"))
            nc.vector.tensor_copy(kjb, kjf)

            # y^T buffers, one per sample: [r=32 part, padded t] in bf16
            ysb = []
            for b in range(BPC):
                y = keep.tile([R, YW], BF16, name=f"ysb{b}")
                nc.vector.memset(y[:, 0:PAD], 0.0)
                nc.vector.memset(y[:, YW - PAD:YW], 0.0)
                ysb.append(y)

            # proj for this core's samples: [nc%128 part, (b, ch), r], bf16
            projw = keep.tile([128, BPC * CH, R], BF16)

            # AllToAll bounce buffers: [src/dst core, bn rows, W2-col slice]
            in_b = dram.tile([NCORES, BPC * N, W2C], BF16)
            out_b = dram.tile([NCORES, BPC * N, W2C], BF16)

            # ---- phase A: proj slices for all samples, AllToAll, reshard ----
            with tc.tile_pool(name="pA", bufs=1) as pA, \
                 tc.tile_pool(name="pAs", bufs=2) as pAs, \
                 tc.tile_pool(name="psA", bufs=2, space="PSUM") as psA, \
                 tc.high_priority():
                # combined^T [c2%128 part, ko, bn] for ALL bn rows, bf16
                phsb = pA.tile([128, 2, C], F32)
                nc.sync.dma_start(phsb, phrase[:, :].rearrange("(rt p) c -> p rt c", p=128))
                eossb = pA.tile([128, 2], F32)
                nc.sync.dma_start(eossb, eos[:].rearrange("(o p) -> p o", p=128))
                combT = pA.tile([128, 4, BN], BF16)
                for rt in range(2):
                    for ko in range(2):
                        pht = psA.tile([128, 128], F32, tag="ph")
                        nc.tensor.transpose(pht, phsb[:, rt, ko * 128:(ko + 1) * 128],
                                            ident)
                        nc.vector.tensor_copy(combT[:, ko, rt * 128:(rt + 1) * 128], pht)
                for o in range(2):
                    nc.vector.tensor_copy(
                        combT[:, 2 + o, :],
                        eossb[:, o:o + 1].to_broadcast((128, BN)))

                # W1 -> bf16 [c2%128, ko, m]; b1 -> [m%128, mo]
                w1sb = pA.tile([128, 4, 4 * C], F32)
                nc.sync.dma_start(w1sb, w1[:, :].rearrange("(ko p) m -> p ko m", p=128))
                w1b = pA.tile([128, 4, 4 * C], BF16)
                nc.vector.tensor_copy(w1b, w1sb)
                b1sb = pA.tile([128, 8], F32)
                nc.sync.dma_start(b1sb, b1[:].rearrange("(mo p) -> p mo", p=128))

                # h^T [m%128 part, mo, bn] = relu(W1^T combined + b1), bf16
                hT = pA.tile([128, 8, BN], BF16)
                for mo in range(8):
                    phm = psA.tile([128, BN], F32, tag="h")
                    for ko in range(4):
                        nc.tensor.matmul(phm, w1b[:, ko, mo * 128:(mo + 1) * 128],
                                         combT[:, ko, :],
                                         start=(ko == 0), stop=(ko == 3))
                    nc.scalar.activation(out=hT[:, mo, :], in_=phm, func=RELU,
                                         bias=b1sb[:, mo:mo + 1], scale=1.0)

                # W2 slice -> bf16, streamed in quarters; b2 -> bf16
                w2b = pA.tile([128, 8, W2C], BF16)
                for qw in range(4):
                    w2q = pA.tile([128, 2, W2C], F32, tag="w2q", bufs=1, name=f"w2q{qw}")
                    nc.sync.dma_start(
                        w2q, w2s[qw * 256:(qw + 1) * 256, :]
                        .rearrange("(ko p) q -> p ko q", p=128))
                    nc.gpsimd.tensor_copy(w2b[:, 2 * qw:2 * qw + 2, :], w2q)
                b2f = pA.tile([1, W2C], F32)
                nc.sync.dma_start(b2f, b2s[:].rearrange("(p q) -> p q", p=1))
                b2b = pA.tile([1, W2C], BF16)
                nc.vector.tensor_copy(b2b, b2f)

                # proj slice rows [bn, W2C] for all samples; +b2 via rank-1 matmul
                for rt in range(2):
                    for cc in range(2):
                        ppp = psA.tile([128, 512], F32, tag="pj")
                        for ko in range(8):
                            nc.tensor.matmul(
                                ppp, hT[:, ko, rt * 128:(rt + 1) * 128],
                                w2b[:, ko, cc * 512:(cc + 1) * 512],
                                start=(ko == 0), stop=False)
                        nc.tensor.matmul(ppp, ones1[:, 0:128],
                                         b2b[:, cc * 512:(cc + 1) * 512],
                                         start=False, stop=True)
                        ppsb = pAs.tile([128, 512], BF16, tag="ppsb")
                        nc.scalar.copy(ppsb, ppp)
                        nc.scalar.dma_start(
                            in_b[rt * 4:(rt + 1) * 4, :, cc * 512:(cc + 1) * 512], ppsb)

                nc.gpsimd.collective_compute(
                    "AllToAll",
                    mybir.AluOpType.bypass,
                    replica_groups=[list(range(NCORES))],
                    ins=[in_b[:, :, :].opt()],
                    outs=[out_b[:, :, :].opt()],
                )

                # reshard: out_b[i = half*4+q, b*16+n, cl*32+r] -> projw[q*32+cl, (b,ch), r]
                # with ch = n*2 + half, i.e. nc = ch*128 + p matching xT's chunks.
                ob = out_b[:, :, :]
                for q in range(4):
                    for half in range(2):
                        src = bass.AP(
                            tensor=ob.tensor,
                            offset=ob.offset + (half * 4 + q) * (BPC * N * W2C),
                            ap=[[32, 32],        # cl -> partition within the q-group
                                [W2C, BPC * N],  # (b, n) merged
                                [1, R]])         # r
                        dst = projw[q * 32:(q + 1) * 32, :, :].rearrange(
                            "p (bn two) r -> p two bn r", two=2)[:, half, :, :]
                        nc.scalar.dma_start(dst, src)

            # ---- phase X: per (sample, t-chunk) stream ------------------------
            with tc.tile_pool(name="yp", bufs=2, space="PSUM") as yp, \
                 tc.tile_pool(name="fp", bufs=2, space="PSUM") as fp, \
                 tc.tile_pool(name="op", bufs=2, space="PSUM") as op, \
                 tc.tile_pool(name="pXs", bufs=2) as pXs:
                def emit_feat(b, tq):
                    t0 = tq * TCHUNK
                    # stage 4: feat^T[d-block, t] = sum_j kjoin_j^T @ shift(y^T)
                    featT = pX.tile([128, 6, TCHUNK], BF16, tag="featT")
                    jj = 0
                    for kb, (k, js) in enumerate(KJ):
                        pad = k // 2
                        for dc in range(2):
                            pf = fp.tile([128, TCHUNK], F32, tag="f")
                            for ji, j in enumerate(js):
                                ys = ysb[b][:, PAD + t0 + j - pad:
                                            PAD + t0 + j - pad + TCHUNK]
                                nc.tensor.matmul(
                                    pf, kjb[:, jj + ji, dc * 128:(dc + 1) * 128],
                                    ys, start=(ji == 0), stop=(ji == len(js) - 1))
                            nc.vector.tensor_copy(featT[:, kb * 2 + dc, :], pf)
                        jj += len(js)

                    # stage 5: out[t%128, co] = feat @ Wo, then LN + relu
                    for ts in range(TCHUNK // 128):
                        po = op.tile([128, C], F32, tag="o")
                        for fc in range(6):
                            nc.tensor.matmul(
                                po, featT[:, fc, ts * 128:(ts + 1) * 128],
                                wob[:, fc, :], start=(fc == 0), stop=(fc == 5))
                        osb = pXs.tile([128, C], F32, tag="osb")
                        nc.vector.tensor_add(osb, po, bosb)
                        st = pXs.tile([128, 6], F32, tag="st")
                        nc.vector.bn_stats(out=st, in_=osb)
                        mv = pXs.tile([128, 2], F32, tag="mv")
                        nc.vector.bn_aggr(out=mv, in_=st)
                        # rstd = 1/sqrt(var + eps)
                        rs = pXs.tile([128, 1], F32, tag="rs")
                        nc.scalar.activation(out=rs, in_=mv[:, 1:2], func=SQRT,
                                             bias=eps, scale=1.0)
                        nc.vector.reciprocal(rs, rs)
                        nc.vector.tensor_scalar(osb, osb, mv[:, 0:1], rs,
                                                mybir.AluOpType.subtract,
                                                mybir.AluOpType.mult)
                        nc.vector.tensor_mul(osb, osb, gsb)
                        nc.vector.tensor_add(osb, osb, bsb)
                        nc.vector.tensor_scalar_max(osb, osb, 0.0)
                        nc.sync.dma_start(
                            out[b, t0 + ts * 128:t0 + (ts + 1) * 128, :], osb)

                for b in range(BPC):
                    for tq in range(TQ):
                        chunk = b * TQ + tq
                        t0 = tq * TCHUNK
                        # x natural [t%128, nc] tiles, PE-transposed to [nc%128, ch, t]
                        xT = pX.tile([128, CH, TCHUNK], BF16, tag="xT")
                        for ts in range(TCHUNK // 128):
                            xn = pXn.tile([128, NCF], F32, tag="xn")
                            nc.sync.dma_start(
                                xn, xb[b, t0 + ts * 128:t0 + (ts + 1) * 128, :])
                            for cg in range(CH // 4):   # 4 transposes per bank
                                pt = tp.tile([128, 4, 128], F32, tag="tp")
                                for q in range(4):
                                    chx = cg * 4 + q
                                    nc.tensor.transpose(
                                        pt[:, q, :],
                                        xn[:, chx * 128:(chx + 1) * 128], ident)
                                dst = xT[:, cg * 4:(cg + 1) * 4,
                                         ts * 128:(ts + 1) * 128]
                                # first chunks: DVE only (scalar is busy with
                                # phase A's bounce DMAs until the A2A lands)
                                if chunk < 2 or cg % 2 == 0:
                                    nc.vector.tensor_copy(dst, pt)
                                else:
                                    nc.scalar.copy(dst, pt)

                        # stage 3: y^T[r, t-chunk] = sum_ch proj^T @ xT
                        py = yp.tile([R, TCHUNK], F32, tag="y")
                        for ch in range(CH):
                            nc.tensor.matmul(py, projw[:, b * CH + ch, :],
                                             xT[:, ch, :],
                                             start=(ch == 0), stop=(ch == CH - 1))
                        nc.vector.tensor_copy(ysb[b][:, PAD + t0:PAD + t0 + TCHUNK],
                                              py)
                        # stage 4/5 lag one chunk: feat(tq-1) needs y[tq]'s
                        # first PAD columns (k=5 right overhang)
                        if tq > 0:
                            emit_feat(b, tq - 1)
                    emit_feat(b, TQ - 1)

    nc.compile()
    return nc


_NC = None


def _get_nc():
    global _NC
    if _NC is None:
        _NC = _build()
    return _NC


def _shard(inputs):
    """Split full inputs into per-core input maps (pure slicing/stacking)."""
    x = np.ascontiguousarray(inputs["context_emb"], dtype=np.float32)
    B = x.shape[0]
    assert B == NCORES * BPC
    x = x.reshape(B, T, NCF)
    ph = np.ascontiguousarray(
        inputs["phrase_slot"], dtype=np.float32).reshape(BN, C)
    w2 = np.asarray(inputs["W2"], dtype=np.float32)
    b2 = np.asarray(inputs["b2"], dtype=np.float32)
    kjoin = np.ascontiguousarray(np.concatenate(
        [np.moveaxis(inputs[f"k{k}"], 2, 0) for k in (1, 3, 5)], axis=0),
        dtype=np.float32)  # [9, 32, 256]
    shared = {
        "phrase": ph,
        "eos": np.ascontiguousarray(inputs["eos_slot"].reshape(C), dtype=np.float32),
        "w1": np.ascontiguousarray(inputs["W1"], dtype=np.float32),
        "b1": np.ascontiguousarray(inputs["b1"], dtype=np.float32),
        "kjoin": kjoin,
        "wo": np.ascontiguousarray(inputs["Wo"], dtype=np.float32),
        "bo": np.ascontiguousarray(inputs["bo"], dtype=np.float32),
        "gamma": np.ascontiguousarray(inputs["gamma"], dtype=np.float32),
        "beta": np.ascontiguousarray(inputs["beta"], dtype=np.float32),
    }
    in_maps = []
    for i in range(NCORES):
        m = dict(shared)
        m["xb"] = np.ascontiguousarray(x[i * BPC:(i + 1) * BPC])
        m["w2s"] = np.ascontiguousarray(w2[:, i * W2C:(i + 1) * W2C])
        m["b2s"] = np.ascontiguousarray(b2[i * W2C:(i + 1) * W2C])
        in_maps.append(m)
    return in_maps


def _run(inputs, **kwargs):
    nc = _get_nc()
    res = run_bass_kernel_spmd(nc, _shard(inputs), core_ids=list(range(NCORES)),
                               **kwargs)
    outs = [r["out"] for r in res.results]
    full = np.concatenate(outs, axis=0).reshape(NCORES * BPC, T, C)
    return full, res


def kernel(**inputs) -> np.ndarray:
    out, _ = _run(inputs)
    return out


# revision 10
# speedup vs baseline: 1.5962x; 1.0674x over previous
"""Trainium2 Bass kernel for nn_LowRankDynamicConv.

Math (per sample b):
  combined = [phrase_slot[b] | eos]                       [N, 2C]
  h        = relu(combined @ W1 + b1)                     [N, 4C]
  proj     = (h @ W2 + b2) viewed as [N*C, R]             [4096, 32]
  y        = x[b] @ proj   with x[b] = context_emb[b] as  [T, N*C]
  out_k[t] = sum_j y[t + j - pad_k] @ kparam_k[:, :, j]   [T, C] for k in (1,3,5)
  out      = relu(LN(concat(out_k) @ Wo + bo))            [T, C]

This is the low-rank refactor of the reference's dense dynamic conv:
  out_k = sum_j shift_j(x) @ (proj @ kparam_k[:,:,j]) == sum_j shift_j(x @ proj) @ kparam_k

Sharding: data-parallel over batch, 2 samples per core. W2 (the one big
weight, 32MB) is column-sharded: every core computes the proj slice for
ALL 16 samples with its 1/8 of W2's columns, then an on-chip AllToAll
redistributes so each core holds the full proj for its own 2 samples.
This cuts per-core HBM traffic by ~28MB vs replicating W2.

Precision: weights and activations feeding matmuls are cast to bf16
(fast weight load + 1 cyc/row); accumulation stays f32 in PSUM, and the
LN epilogue is f32. x is PE-transposed in f32 and stored bf16. Measured
end-to-end rel err vs the f32 reference is well under the 2e-2 gate.
"""
import sys

sys.path.insert(0, "/opt/trn_rl_repo")

import numpy as np

import concourse.bass as bass
import concourse.mybir as mybir
import concourse.tile as tile
from concourse import bacc
from concourse.bass_utils import run_bass_kernel_spmd
from concourse.masks import make_identity

F32 = mybir.dt.float32
BF16 = mybir.dt.bfloat16
RELU = mybir.ActivationFunctionType.Relu
SQRT = mybir.ActivationFunctionType.Sqrt

NCORES = 8
BPC = 2                    # samples per core
T, N, C, R = 1024, 16, 256, 32
BN = NCORES * BPC * N      # 256 (b, n) rows across ALL samples
NCF = N * C                # 4096 flattened (n, c) contraction dim
CH = NCF // 128            # 32 nc-chunks of 128
TQ = 2                     # t processed in 2 chunks of 512
TCHUNK = T // TQ           # 512
PAD = 2                    # max conv pad (k=5)
YW = T + 2 * PAD           # padded y width, 1028
W2C = C * R // NCORES      # 1024 W2 columns per core
# (kernel_size, j) pairs in feat-concat order: k1 | k3 | k5
KJ = [(1, [0]), (3, [0, 1, 2]), (5, [0, 1, 2, 3, 4])]
NJ = 9                     # total j count


def _broadcast_ap(ap, parts):
    """DMA access pattern replicating a 1D/2D DRAM tensor across `parts` partitions."""
    a = ap
    return bass.AP(tensor=a.tensor, offset=a.offset, ap=[[0, parts]] + list(a.ap))


def _build():
    nc = bacc.Bacc("TRN2", num_devices=NCORES)

    xb = nc.dram_tensor("xb", [BPC, T, NCF], F32, kind="ExternalInput")
    phrase = nc.dram_tensor("phrase", [BN, C], F32, kind="ExternalInput")
    eos = nc.dram_tensor("eos", [C], F32, kind="ExternalInput")
    w1 = nc.dram_tensor("w1", [2 * C, 4 * C], F32, kind="ExternalInput")
    b1 = nc.dram_tensor("b1", [4 * C], F32, kind="ExternalInput")
    w2s = nc.dram_tensor("w2s", [4 * C, W2C], F32, kind="ExternalInput")
    b2s = nc.dram_tensor("b2s", [W2C], F32, kind="ExternalInput")
    kjoin = nc.dram_tensor("kjoin", [NJ, R, C], F32, kind="ExternalInput")
    wo = nc.dram_tensor("wo", [3 * C, C], F32, kind="ExternalInput")
    bo = nc.dram_tensor("bo", [C], F32, kind="ExternalInput")
    gamma = nc.dram_tensor("gamma", [C], F32, kind="ExternalInput")
    beta = nc.dram_tensor("beta", [C], F32, kind="ExternalInput")
    out = nc.dram_tensor("out", [BPC, T, C], F32, kind="ExternalOutput")

    with tile.TileContext(nc) as tc:
        with tc.tile_pool(name="keep", bufs=1) as keep, \
             tc.tile_pool(name="dram", bufs=1, space="DRAM") as dram, \
             tc.tile_pool(name="tp", bufs=2, space="PSUM") as tp, \
             tc.tile_pool(name="pXn", bufs=2) as pXn, \
             tc.tile_pool(name="pX", bufs=2) as pX:
            ident = keep.tile([128, 128], F32)
            make_identity(nc, ident)
            ones1 = keep.tile([1, 128], BF16)
            nc.vector.memset(ones1, 1.0)
            eps = keep.tile([128, 1], F32)
            nc.vector.memset(eps, 1e-5)

            # LN params + output bias, broadcast across partitions
            gsb = keep.tile([128, C], F32)
            nc.sync.dma_start(gsb, _broadcast_ap(gamma[:], 128))
            bsb = keep.tile([128, C], F32)
            nc.sync.dma_start(bsb, _broadcast_ap(beta[:], 128))
            bosb = keep.tile([128, C], F32)
            nc.sync.dma_start(bosb, _broadcast_ap(bo[:], 128))

            # bf16 stage-4/5 weights, staged through one shared f32 scratch tag
            wob = keep.tile([128, 6, C], BF16)
            wof = keep.tile([128, 6, C], F32, tag="stg", name="wof")
            nc.sync.dma_start(wof, wo[:, :].rearrange("(fc p) co -> p fc co", p=128))
            nc.vector.tensor_copy(wob, wof)
            kjb = keep.tile([R, NJ, C], BF16)
            kjf = keep.tile([R, NJ, C], F32, tag="stg", name="kjf")
            nc.sync.dma_start(kjf, kjoin[:, :, :].rearrange("j r d -> r j d"))
            nc.vector.tensor_copy(kjb, kjf)

            # y^T buffers, one per sample: [r=32 part, padded t] in bf16
            ysb = []
            for b in range(BPC):
                y = keep.tile([R, YW], BF16, name=f"ysb{b}")
                nc.vector.memset(y[:, 0:PAD], 0.0)
                nc.vector.memset(y[:, YW - PAD:YW], 0.0)
                ysb.append(y)

            # proj for this core's samples: [nc%128 part, (b, ch), r], bf16
            projw = keep.tile([128, BPC * CH, R], BF16)

            # AllToAll bounce buffers: [src/dst core, bn rows, W2-col slice]
            in_b = dram.tile([NCORES, BPC * N, W2C], BF16)
            out_b = dram.tile([NCORES, BPC * N, W2C], BF16)

            # ---- phase A: proj slices for all samples, AllToAll, reshard ----
            with tc.tile_pool(name="pA", bufs=1) as pA, \
                 tc.tile_pool(name="pAs", bufs=2) as pAs, \
                 tc.tile_pool(name="psA", bufs=2, space="PSUM") as psA, \
                 tc.high_priority():
                # combined^T [c2%128 part, ko, bn] for ALL bn rows, bf16
                phsb = pA.tile([128, 2, C], F32)
                nc.sync.dma_start(phsb, phrase[:, :].rearrange("(rt p) c -> p rt c", p=128))
                eossb = pA.tile([128, 2], F32)
                nc.sync.dma_start(eossb, eos[:].rearrange("(o p) -> p o", p=128))
                combT = pA.tile([128, 4, BN], BF16)
                for rt in range(2):
                    for ko in range(2):
                        pht = psA.tile([128, 128], F32, tag="h")
                        nc.tensor.transpose(pht, phsb[:, rt, ko * 128:(ko + 1) * 128],
                                            ident)
                        nc.vector.tensor_copy(combT[:, ko, rt * 128:(rt + 1) * 128], pht)
                for o in range(2):
                    nc.vector.tensor_copy(
                        combT[:, 2 + o, :],
                        eossb[:, o:o + 1].to_broadcast((128, BN)))

                # W1 -> bf16 [c2%128, ko, m]; b1 -> [m%128, mo]
                w1sb = pA.tile([128, 4, 4 * C], F32)
                nc.sync.dma_start(w1sb, w1[:, :].rearrange("(ko p) m -> p ko m", p=128))
                w1b = pA.tile([128, 4, 4 * C], BF16)
                nc.vector.tensor_copy(w1b, w1sb)
                b1sb = pA.tile([128, 8], F32)
                nc.sync.dma_start(b1sb, b1[:].rearrange("(mo p) -> p mo", p=128))

                # h^T [m%128 part, mo, bn] = relu(W1^T combined + b1), bf16
                hT = pA.tile([128, 8, BN], BF16)
                for mo in range(8):
                    phm = psA.tile([128, BN], F32, tag="h")
                    for ko in range(4):
                        nc.tensor.matmul(phm, w1b[:, ko, mo * 128:(mo + 1) * 128],
                                         combT[:, ko, :],
                                         start=(ko == 0), stop=(ko == 3))
                    nc.scalar.activation(out=hT[:, mo, :], in_=phm, func=RELU,
                                         bias=b1sb[:, mo:mo + 1], scale=1.0)

                # W2 slice -> bf16, streamed in quarters; b2 -> bf16
                w2b = pA.tile([128, 8, W2C], BF16)
                for qw in range(4):
                    w2q = pA.tile([128, 2, W2C], F32, tag="w2q", bufs=2, name=f"w2q{qw}")
                    nc.sync.dma_start(
                        w2q, w2s[qw * 256:(qw + 1) * 256, :]
                        .rearrange("(ko p) q -> p ko q", p=128))
                    nc.vector.tensor_copy(w2b[:, 2 * qw, :], w2q[:, 0, :])
                    nc.scalar.copy(w2b[:, 2 * qw + 1, :], w2q[:, 1, :])
                b2f = keep.tile([1, W2C], F32, tag="stg", name="b2f")
                nc.sync.dma_start(b2f, b2s[:].rearrange("(p q) -> p q", p=1))
                b2b = pA.tile([1, W2C], BF16)
                nc.vector.tensor_copy(b2b, b2f)

                # proj slice rows [bn, W2C] for all samples; +b2 via rank-1 matmul
                for rt in range(2):
                    for cc in range(2):
                        ppp = psA.tile([128, 512], F32, tag="pj", bufs=4)
                        for ko in range(8):
                            nc.tensor.matmul(
                                ppp, hT[:, ko, rt * 128:(rt + 1) * 128],
                                w2b[:, ko, cc * 512:(cc + 1) * 512],
                                start=(ko == 0), stop=False)
                        nc.tensor.matmul(ppp, ones1[:, 0:128],
                                         b2b[:, cc * 512:(cc + 1) * 512],
                                         start=False, stop=True)
                        ppsb = pAs.tile([128, 512], BF16, tag="ppsb")
                        nc.scalar.copy(ppsb, ppp)
                        nc.scalar.dma_start(
                            in_b[rt * 4:(rt + 1) * 4, :, cc * 512:(cc + 1) * 512], ppsb)

                nc.gpsimd.collective_compute(
                    "AllToAll",
                    mybir.AluOpType.bypass,
                    replica_groups=[list(range(NCORES))],
                    ins=[in_b[:, :, :].opt()],
                    outs=[out_b[:, :, :].opt()],
                )

                # reshard: out_b[i = half*4+q, b*16+n, cl*32+r] -> projw[q*32+cl, (b,ch), r]
                # with ch = n*2 + half, i.e. nc = ch*128 + p matching xT's chunks.
                ob = out_b[:, :, :]
                for q in range(4):
                    for half in range(2):
                        src = bass.AP(
                            tensor=ob.tensor,
                            offset=ob.offset + (half * 4 + q) * (BPC * N * W2C),
                            ap=[[32, 32],        # cl -> partition within the q-group
                                [W2C, BPC * N],  # (b, n) merged
                                [1, R]])         # r
                        dst = projw[q * 32:(q + 1) * 32, :, :].rearrange(
                            "p (bn two) r -> p two bn r", two=2)[:, half, :, :]
                        nc.scalar.dma_start(dst, src)

            # ---- phase X: per (sample, t-chunk) stream ------------------------
            with tc.tile_pool(name="yp", bufs=2, space="PSUM") as yp, \
                 tc.tile_pool(name="fp", bufs=2, space="PSUM") as fp, \
                 tc.tile_pool(name="op", bufs=2, space="PSUM") as op, \
                 tc.tile_pool(name="pXs", bufs=2) as pXs:
                def emit_feat(b, tq):
                    t0 = tq * TCHUNK
                    # stage 4: feat^T[d-block, t] = sum_j kjoin_j^T @ shift(y^T)
                    featT = pX.tile([128, 6, TCHUNK], BF16, tag="featT")
                    jj = 0
                    for kb, (k, js) in enumerate(KJ):
                        pad = k // 2
                        for dc in range(2):
                            pf = fp.tile([128, TCHUNK], F32, tag="f")
                            for ji, j in enumerate(js):
                                ys = ysb[b][:, PAD + t0 + j - pad:
                                            PAD + t0 + j - pad + TCHUNK]
                                nc.tensor.matmul(
                                    pf, kjb[:, jj + ji, dc * 128:(dc + 1) * 128],
                                    ys, start=(ji == 0), stop=(ji == len(js) - 1))
                            nc.vector.tensor_copy(featT[:, kb * 2 + dc, :], pf)
                        jj += len(js)

                    # stage 5: out[t%128, co] = feat @ Wo, then LN + relu
                    for ts in range(TCHUNK // 128):
                        po = op.tile([128, C], F32, tag="o")
                        for fc in range(6):
                            nc.tensor.matmul(
                                po, featT[:, fc, ts * 128:(ts + 1) * 128],
                                wob[:, fc, :], start=(fc == 0), stop=(fc == 5))
                        osb = pXs.tile([128, C], F32, tag="osb")
                        nc.vector.tensor_add(osb, po, bosb)
                        st = pXs.tile([128, 6], F32, tag="st")
                        nc.vector.bn_stats(out=st, in_=osb)
                        mv = pXs.tile([128, 2], F32, tag="mv")
                        nc.vector.bn_aggr(out=mv, in_=st)
                        # rstd = 1/sqrt(var + eps)
                        rs = pXs.tile([128, 1], F32, tag="rs")
                        nc.scalar.activation(out=rs, in_=mv[:, 1:2], func=SQRT,
                                             bias=eps, scale=1.0)
                        nc.vector.reciprocal(rs, rs)
                        nc.vector.tensor_scalar(osb, osb, mv[:, 0:1], rs,
                                                mybir.AluOpType.subtract,
                                                mybir.AluOpType.mult)
                        nc.vector.tensor_mul(osb, osb, gsb)
                        nc.vector.tensor_add(osb, osb, bsb)
                        nc.vector.tensor_scalar_max(osb, osb, 0.0)
                        nc.sync.dma_start(
                            out[b, t0 + ts * 128:t0 + (ts + 1) * 128, :], osb)

                for b in range(BPC):
                    for tq in range(TQ):
                        chunk = b * TQ + tq
                        t0 = tq * TCHUNK
                        # x natural [t%128, nc] tiles, PE-transposed to [nc%128, ch, t]
                        xT = pX.tile([128, CH, TCHUNK], BF16, tag="xT")
                        for ts in range(TCHUNK // 128):
                            xn = pXn.tile([128, NCF], F32, tag="xn")
                            nc.sync.dma_start(
                                xn, xb[b, t0 + ts * 128:t0 + (ts + 1) * 128, :])
                            for cg in range(CH // 4):   # 4 transposes per bank
                                pt = tp.tile([128, 4, 128], F32, tag="tp")
                                for q in range(4):
                                    chx = cg * 4 + q
                                    nc.tensor.transpose(
                                        pt[:, q, :],
                                        xn[:, chx * 128:(chx + 1) * 128], ident)
                                dst = xT[:, cg * 4:(cg + 1) * 4,
                                         ts * 128:(ts + 1) * 128]
                                # first chunks: DVE only (scalar is busy with
                                # phase A's bounce DMAs until the A2A lands)
                                if chunk < 2 or cg % 2 == 0:
                                    nc.vector.tensor_copy(dst, pt)
                                else:
                                    nc.scalar.copy(dst, pt)

                        # stage 3: y^T[r, t-chunk] = sum_ch proj^T @ xT
                        py = yp.tile([R, TCHUNK], F32, tag="y")
                        for ch in range(CH):
                            nc.tensor.matmul(py, projw[:, b * CH + ch, :],
                                             xT[:, ch, :],
                                             start=(ch == 0), stop=(ch == CH - 1))
                        nc.vector.tensor_copy(ysb[b][:, PAD + t0:PAD + t0 + TCHUNK],
                                              py)
                        # stage 4/5 lag one chunk: feat(tq-1) needs y[tq]'s
                        # first PAD columns (k=5 right overhang)
                        if tq > 0:
                            emit_feat(b, tq - 1)
                    emit_feat(b, TQ - 1)

    nc.compile()
    return nc


_NC = None


def _get_nc():
    global _NC
    if _NC is None:
        _NC = _build()
    return _NC


def _shard(inputs):
    """Split full inputs into per-core input maps (pure slicing/stacking)."""
    x = np.ascontiguousarray(inputs["context_emb"], dtype=np.float32)
    B = x.shape[0]
    assert B == NCORES * BPC
    x = x.reshape(B, T, NCF)
    ph = np.ascontiguousarray(
        inputs["phrase_slot"], dtype=np.float32).reshape(BN, C)
    w2 = np.asarray(inputs["W2"], dtype=np.float32)
    b2 = np.asarray(inputs["b2"], dtype=np.float32)
    kjoin = np.ascontiguousarray(np.concatenate(
        [np.moveaxis(inputs[f"k{k}"], 2, 0) for k in (1, 3, 5)], axis=0),
        dtype=np.float32)  # [9, 32, 256]
    shared = {
        "phrase": ph,
        "eos": np.ascontiguousarray(inputs["eos_slot"].reshape(C), dtype=np.float32),
        "w1": np.ascontiguousarray(inputs["W1"], dtype=np.float32),
        "b1": np.ascontiguousarray(inputs["b1"], dtype=np.float32),
        "kjoin": kjoin,
        "wo": np.ascontiguousarray(inputs["Wo"], dtype=np.float32),
        "bo": np.ascontiguousarray(inputs["bo"], dtype=np.float32),
        "gamma": np.ascontiguousarray(inputs["gamma"], dtype=np.float32),
        "beta": np.ascontiguousarray(inputs["beta"], dtype=np.float32),
    }
    in_maps = []
    for i in range(NCORES):
        m = dict(shared)
        m["xb"] = np.ascontiguousarray(x[i * BPC:(i + 1) * BPC])
        m["w2s"] = np.ascontiguousarray(w2[:, i * W2C:(i + 1) * W2C])
        m["b2s"] = np.ascontiguousarray(b2[i * W2C:(i + 1) * W2C])
        in_maps.append(m)
    return in_maps


def _run(inputs, **kwargs):
    nc = _get_nc()
    res = run_bass_kernel_spmd(nc, _shard(inputs), core_ids=list(range(NCORES)),
                               **kwargs)
    outs = [r["out"] for r in res.results]
    full = np.concatenate(outs, axis=0).reshape(NCORES * BPC, T, C)
    return full, res


def kernel(**inputs) -> np.ndarray:
    out, _ = _run(inputs)
    return out
